# revision 1
# baseline (speedup 1.0000x reference)
"""AttentionRPE kernel for 8 Trainium2 NeuronCores.

Math (per (b,s) row, T=128 targets, D=256, H=8 heads, DH=32, DR=32):
  q   = src @ Wsrc.T + bsrc                       [D]
  K'  = tgt @ Wk.T + rpe @ Rwk.T (+const bias)    [T, D]
  V'  = tgt @ Wv.T + rpe @ Rwv.T (+const bias)    [T, D]
  att = softmax_h(q_h . K'_h / sqrt(DH))          [H, T]   (masked)
  out = (att @ V')_heads @ Wout.T + bout          [D]

Device formulation (the tricks that make it fast):
  * K-path q-fold: logits[h,t] = sum_d qw[h,d]*tgtx[t,d], with
    qw = (q/sqrt(DH)) @ Wkx  folded per row (tiny), tgtx = [tgt | rpe]
    ([T, 288]).  Only tgtx must be transposed on-chip (PE transposes).
  * V-path commute: G[d,h] = sum_t tgtx[t,d]*att[h,t] first (natural
    layout matmul), then out = sum_{h,d} G[d,h]*Wfx[h,d,:] where
    Wfx[h] = (Wout[:,hslice] @ Wvx[hslice,:]).T is precomputed on host.
    The V/rpe_v projection of the big tensor is never materialized.
  * Padding mask + off-diagonal garbage masking folded into one extra
    accumulating matmul into the logits PSUM (rank-16 selector A16 @ Bm).
  * All biases except bsrc are constant in t and either cancel in
    softmax (K-side) or fold into the output bias (V-side).

Sharding: 1024 (b,s) rows split contiguously over 8 cores (128 each).
"""

import numpy as np

import concourse.bass as bass
import concourse.bacc as bacc
import concourse.mybir as mybir
from concourse.tile import TileContext
from concourse.masks import make_identity
from concourse.bass_utils import run_bass_kernel_spmd

B, S, T, D = 2, 512, 128, 256
H, DH, DR = 8, 32, 32
DX = D + DR          # 288 = tgt|rpe feature dim
DOUT = D
NCORES = 8
BS = B * S           # 1024 total rows
SC = BS // NCORES    # 128 rows per core

F32 = mybir.dt.float32
F32R = mybir.dt.float32r

AX = mybir.AxisListType
ALU = mybir.AluOpType
ACTF = mybir.ActivationFunctionType

# float32r (full-rate, reduced-precision fp32) per matmul group.
R_MASK = True      # mask add matmul (values 0/-1e30: always safe)
R_TRANSP = False  # (transposes stay exact fp32)
R_LOGITS = True   # (via float32r-typed operand tiles)
R_GT = False       # G = att @ tgtx matmuls
R_FINAL = True    # output projection matmuls
R_QPATH = False    # q / qw / qrw setup matmuls


def _r(ap, on):
    return ap.bitcast(F32R) if on else ap


def build(sc=SC):
    """Build the per-core Bass program. sc = rows per core (multiple of 16)."""
    assert sc % 16 == 0
    nblk = sc // 16
    nc = bacc.Bacc()

    src_d = nc.dram_tensor("src", [sc, D], F32, kind="ExternalInput")
    tgtx_d = nc.dram_tensor("tgtx", [sc, T, DX], F32, kind="ExternalInput")
    bm_d = nc.dram_tensor("bm", [nblk, 16, 512], F32, kind="ExternalInput")
    a16x_d = nc.dram_tensor("a16x", [16, sc, H], F32, kind="ExternalInput")
    wsrcT_d = nc.dram_tensor("wsrcT", [D, D], F32, kind="ExternalInput")
    wk_d = nc.dram_tensor("wk", [D, D], F32, kind="ExternalInput")
    rwk_d = nc.dram_tensor("rwk", [D, DR], F32, kind="ExternalInput")
    wfx_d = nc.dram_tensor("wfx", [H, DX, DOUT], F32R, kind="ExternalInput")
    bsrc_d = nc.dram_tensor("bsrc", [D, 1], F32, kind="ExternalInput")
    obias_d = nc.dram_tensor("obias", [sc, DOUT], F32, kind="ExternalInput")
    rmask_d = nc.dram_tensor("rmask", [sc, 1], F32, kind="ExternalInput")
    out_d = nc.dram_tensor("out", [sc, DOUT], F32, kind="ExternalOutput")

    with TileContext(nc) as tc:
        with (
            tc.tile_pool(name="const", bufs=1) as cp,
            tc.tile_pool(name="txp", bufs=10) as txp,
            tc.tile_pool(name="txtp", bufs=3) as txtp,
            tc.tile_pool(name="attnp", bufs=2) as attnp,
            tc.tile_pool(name="smallp", bufs=2) as smallp,
            tc.tile_pool(name="ps_tr", bufs=1, space="PSUM") as ps_tr,
            tc.tile_pool(name="ps_tr2", bufs=1, space="PSUM") as ps_tr2,
            tc.tile_pool(name="ps_l", bufs=2, space="PSUM") as ps_l,
            tc.tile_pool(name="ps_misc", bufs=1, space="PSUM") as ps_misc,
        ):
            # ---------------- constants & weights ----------------
            eye = cp.tile([128, 128], F32, name="eye")
            make_identity(nc, eye)
            wsrcT = []
            wk = []
            rwk = []
            for c in range(2):
                w1 = cp.tile([128, D], F32, name=f"wsrcT{c}")
                nc.sync.dma_start(out=w1, in_=wsrcT_d[c * 128:(c + 1) * 128, :])
                wsrcT.append(w1)
                w2 = cp.tile([128, D], F32, name=f"wk{c}")
                nc.sync.dma_start(out=w2, in_=wk_d[c * 128:(c + 1) * 128, :])
                wk.append(w2)
                w3 = cp.tile([128, DR], F32, name=f"rwk{c}")
                nc.sync.dma_start(out=w3, in_=rwk_d[c * 128:(c + 1) * 128, :])
                rwk.append(w3)
            wfx_main = {}
            wfx_r = {}
            for h in range(H):
                for c in range(2):
                    wt = cp.tile([128, DOUT], F32R, name=f"wfx{h}_{c}")
                    nc.sync.dma_start(
                        out=wt, in_=wfx_d[h, c * 128:(c + 1) * 128, :])
                    wfx_main[(h, c)] = wt
                wt = cp.tile([32, DOUT], F32R, name=f"wfxr{h}")
                nc.sync.dma_start(out=wt, in_=wfx_d[h, D:DX, :])
                wfx_r[h] = wt
            bsrc = []
            for c in range(2):
                bt = cp.tile([128, 1], F32, name=f"bsrc{c}")
                nc.sync.dma_start(out=bt, in_=bsrc_d[c * 128:(c + 1) * 128, :])
                bsrc.append(bt)
            obias = cp.tile([sc, DOUT], F32, name="obias")
            nc.sync.dma_start(out=obias, in_=obias_d[:, :])
            rmask = cp.tile([sc, 1], F32, name="rmask")
            nc.sync.dma_start(out=rmask, in_=rmask_d[:, :])

            gall = cp.tile([128, sc, 24], F32R, name="gall")

            # ---------------- q path (once per core) ----------------
            src_sb = cp.tile([sc, D], F32, name="src_sb")
            nc.sync.dma_start(out=src_sb, in_=src_d[:, :])
            srcT = []
            for c in range(2):
                st_ps = ps_misc.tile([128, sc], F32, tag="mA", name="st_ps", bufs=2)
                nc.tensor.transpose(st_ps, src_sb[:, c * 128:(c + 1) * 128],
                                    eye[0:sc, 0:sc])
                st = cp.tile([128, sc], F32, name=f"srcT{c}")
                nc.vector.tensor_copy(st, st_ps)
                srcT.append(st)
            qT = []
            for ec in range(2):
                q_ps = ps_misc.tile([128, sc], F32, tag="mB", name="q_ps")
                for dc in range(2):
                    nc.tensor.matmul(
                        q_ps,
                        _r(wsrcT[dc][:, ec * 128:(ec + 1) * 128], R_QPATH),
                        _r(srcT[dc], R_QPATH),
                        start=(dc == 0), stop=(dc == 1))
                qt = cp.tile([128, sc], F32, name=f"qT{ec}")
                nc.vector.tensor_scalar_add(qt, q_ps, bsrc[ec])
                qT.append(qt)
            qwT = []
            for dc in range(2):
                qwT.append(cp.tile([128, sc, H], F32, name=f"qwT{dc}"))
            qwTf = [t.rearrange("p s h -> p (s h)") for t in qwT]
            qrwT = cp.tile([48, sc, H], F32, name="qrwT")
            qrwTf = qrwT.rearrange("p s h -> p (s h)")
            nc.sync.dma_start(out=qrwT[32:48, :, :], in_=a16x_d[:, :, :])
            for h in range(H):
                ti, ro = h // 4, (h % 4) * 32
                for dc in range(2):
                    qw_ps = ps_misc.tile([128, sc], F32, tag="mA", name="qw_ps", bufs=2)
                    nc.tensor.matmul(
                        qw_ps,
                        _r(wk[ti][ro:ro + 32, dc * 128:(dc + 1) * 128], R_QPATH),
                        _r(qT[ti][ro:ro + 32, :], R_QPATH),
                        start=True, stop=True, tile_position=(ro, 0))
                    if dc == 0:
                        nc.vector.tensor_copy(qwT[dc][:, :, h], qw_ps)
                    else:
                        nc.scalar.activation(qwT[dc][:, :, h], qw_ps, ACTF.Copy)
                qr_ps = ps_misc.tile([32, sc], F32, tag="mB", name="qr_ps")
                nc.tensor.matmul(
                    qr_ps,
                    _r(rwk[ti][ro:ro + 32, :], R_QPATH),
                    _r(qT[ti][ro:ro + 32, :], R_QPATH),
                    start=True, stop=True, tile_position=(ro, 0))
                nc.vector.tensor_copy(qrwT[0:32, :, h], qr_ps)

            # ---------------- main loop ----------------
            for blk in range(nblk):
                l_ps = ps_l.tile([128, 512], F32, name="l_ps")
                tx_tiles = []
                for g in range(4):
                    c0 = ps_tr.tile([128, 512], F32, tag="c0", name="c0")
                    c1 = ps_tr.tile([128, 512], F32, tag="c1", name="c1")
                    c2 = ps_tr2.tile([32, 512], F32, tag="c2", name="c2")
                    s0 = blk * 16 + g * 4
                    tx4 = txp.tile([T, 4, DX], F32, tag="tx", name="tx4")
                    nc.sync.dma_start(
                        out=tx4, in_=tgtx_d[s0:s0 + 4, :, :].transpose([1, 0, 2]))
                    tx_tiles.append(tx4)
                    for sg in range(4):
                        tx = tx4[:, sg, :]
                        sl = slice(sg * 128, (sg + 1) * 128)
                        nc.tensor.matmul(
                            c0[:, sl], tx[:, 0:128],
                            eye,
                            start=True, stop=True, is_transpose=True)
                        nc.tensor.matmul(
                            c1[:, sl], tx[:, 128:256],
                            eye,
                            start=True, stop=True, is_transpose=True)
                        nc.tensor.matmul(
                            c2[:, sl], tx[:, 256:288],
                            eye,
                            start=True, stop=True, is_transpose=True)
                    t0 = txtp.tile([128, 512], F32, tag="t0", name="t0")
                    t1 = txtp.tile([128, 512], F32, tag="t1", name="t1")
                    t2 = txtp.tile([48, 512], F32, tag="t2", name="t2")
                    nc.sync.dma_start(out=t2[32:48, :], in_=bm_d[blk, :, :])
                    nc.vector.tensor_copy(t0, c0)
                    nc.scalar.activation(t1, c1, ACTF.Copy)
                    nc.vector.tensor_copy(t2[0:32, :], c2)
                    gs = blk * 16 + g * 4
                    osl = slice(g * 32, (g + 1) * 32)
                    nc.tensor.matmul(
                        l_ps[osl, :],
                        qwTf[0][:, gs * 8:gs * 8 + 32],
                        t0, start=True, stop=False,
                        tile_position=(0, g * 32))
                    nc.tensor.matmul(
                        l_ps[osl, :],
                        qwTf[1][:, gs * 8:gs * 8 + 32],
                        t1, start=False, stop=False,
                        tile_position=(0, g * 32))
                    nc.tensor.matmul(
                        l_ps[osl, :],
                        qrwTf[:, gs * 8:gs * 8 + 32],
                        t2, start=False, stop=True,
                        tile_position=(0, g * 32))

                # softmax over the 512-wide rows (off-diag blocks masked to 0)
                nmx = smallp.tile([128, 1], F32, tag="nmx", name="nmx")
                nc.vector.tensor_reduce(nmx, l_ps, axis=AX.X, op=ALU.max,
                                        negate=True)
                den = smallp.tile([128, 1], F32, tag="den", name="den")
                attn_e = attnp.tile([128, 512], F32, tag="ae", name="attn_e")
                nc.scalar.activation(attn_e, l_ps, ACTF.Exp, bias=nmx,
                                     scale=1.0, accum_out=den)
                rden = smallp.tile([128, 1], F32, tag="rden", name="rden")
                nc.vector.reciprocal(rden, den)
                attn_n = attnp.tile([128, 512], F32, tag="an", name="attn_n")
                nc.vector.tensor_scalar_mul(attn_n, attn_e, rden)

                # Off-slot attn entries are exactly 0 (mask -1e30 -> exp -> 0),
                # so the sum of the 4 block transposes is the exact attnT.
                at_ps = ps_misc.tile([128, 128], F32, tag="mA", name="at_ps", bufs=2)
                for g in range(4):
                    nc.tensor.matmul(
                        at_ps,
                        attn_n[:, g * 128:(g + 1) * 128],
                        eye,
                        start=(g == 0), stop=(g == 3), is_transpose=True)
                atT = smallp.tile([128, 128], F32, tag="atT", name="atT")
                nc.vector.tensor_copy(atT, at_ps)

                gt_ps = ps_misc.tile([128, 16, 24], F32, tag="mB", name="gt_ps")
                for j in range(16):
                    tx = tx_tiles[j // 4][:, j % 4, :]
                    av = atT[:, j * 8:(j + 1) * 8]
                    nc.tensor.matmul(gt_ps[:, j, 0:8],
                                     _r(tx[:, 0:128], R_GT), _r(av, R_GT),
                                     start=True, stop=True)
                    nc.tensor.matmul(gt_ps[:, j, 8:16],
                                     _r(tx[:, 128:256], R_GT), _r(av, R_GT),
                                     start=True, stop=True)
                    nc.tensor.matmul(gt_ps[0:32, j, 16:24],
                                     _r(tx[:, 256:288], R_GT), _r(av, R_GT),
                                     start=True, stop=True)
                bsl = slice(blk * 16, (blk + 1) * 16)
                nc.vector.tensor_copy(gall[:, bsl, 0:16], gt_ps[:, :, 0:16])
                nc.scalar.activation(gall[0:32, bsl, 16:24],
                                     gt_ps[0:32, :, 16:24], ACTF.Copy)

            # ---------------- output projection ----------------
            out_ps = ps_misc.tile([sc, DOUT], F32, tag="mA", name="out_ps", bufs=2)
            for h in range(H):
                for c in range(2):
                    nc.tensor.matmul(
                        out_ps,
                        gall[:, :, c * 8 + h],
                        wfx_main[(h, c)],
                        start=(h == 0 and c == 0), stop=False)
                nc.tensor.matmul(
                    out_ps,
                    gall[0:32, :, 16 + h],
                    wfx_r[h],
                    start=False, stop=(h == H - 1))
            out_sb = cp.tile([sc, DOUT], F32, name="out_sb")
            nc.vector.tensor_tensor(out_sb, out_ps, obias, op=ALU.add)
            out_sb2 = cp.tile([sc, DOUT], F32, name="out_sb2")
            nc.vector.tensor_scalar_mul(out_sb2, out_sb, rmask)
            nc.sync.dma_start(out=out_d[:, :], in_=out_sb2)

    nc.finalize()
    return nc


def host_prep(src, tgt, rpe, tgt_padding_mask, in_proj_weight, in_proj_bias,
              out_proj_weight, out_proj_bias, rpe_weight, rpe_bias):
    """Host-side slicing/weight prep. Returns per-core input maps."""
    f = np.float32
    scale = f(1.0 / np.sqrt(DH))
    src_f = np.ascontiguousarray(np.asarray(src, f).reshape(BS, D))
    tgtx = np.concatenate(
        [np.asarray(tgt, f).reshape(BS, T, D),
         np.asarray(rpe, f).reshape(BS, T, DR)], axis=-1)
    mask = np.asarray(tgt_padding_mask, bool).reshape(BS, T)
    no_valid = mask.all(-1)
    maskadd = np.where(mask & ~no_valid[:, None], f(-1e30), f(0.0)).astype(f)
    rowmask = np.ascontiguousarray((~no_valid).astype(f)[:, None])

    nblk_total = BS // 16
    bm = np.full((nblk_total, 16, 4, T), -1e30, f)
    ma_b = maskadd.reshape(nblk_total, 16, T)
    for j in range(16):
        bm[:, j, j % 4, :] = ma_b[:, j, :]
    bm = bm.reshape(nblk_total, 16, 512)
    sidx = np.arange(SC) % 16
    a16x = (np.arange(16)[:, None, None] == sidx[None, :, None]).astype(f)
    a16x = np.ascontiguousarray(np.broadcast_to(a16x, (16, SC, H)))

    ipw = np.asarray(in_proj_weight, f)
    ipb = np.asarray(in_proj_bias, f)
    opw = np.asarray(out_proj_weight, f)
    opb = np.asarray(out_proj_bias, f)
    rw = np.asarray(rpe_weight, f)
    rb = np.asarray(rpe_bias, f)

    wsrcT = np.ascontiguousarray(ipw[:D].T * scale)          # [d, e]
    bsrc = np.ascontiguousarray((ipb[:D] * scale)[:, None])  # [D, 1]
    wk = np.ascontiguousarray(ipw[D:2 * D])                  # [e, d]
    rwk = np.ascontiguousarray(rw[:D])                       # [e, r]
    wvx = np.concatenate([ipw[2 * D:3 * D], rw[D:2 * D]], axis=1)  # [e, 288]
    wfx = np.empty((H, DX, DOUT), f)
    for h in range(H):
        hs = slice(h * 32, (h + 1) * 32)
        wfx[h] = (opw[:, hs] @ wvx[hs, :]).T
    obias = (opb + opw @ (ipb[2 * D:3 * D] + rb[D:2 * D]))[None, :]
    obias = np.ascontiguousarray(np.repeat(obias.astype(f), SC, axis=0))

    wfx = round_f32r(wfx)

    nblk = SC // 16
    in_maps = []
    for c in range(NCORES):
        sl = slice(c * SC, (c + 1) * SC)
        in_maps.append({
            "src": src_f[sl],
            "tgtx": np.ascontiguousarray(tgtx[sl]),
            "bm": np.ascontiguousarray(bm[c * nblk:(c + 1) * nblk]),
            "a16x": a16x,
            "wsrcT": wsrcT,
            "wk": wk,
            "rwk": rwk,
            "wfx": wfx,
            "bsrc": bsrc,
            "obias": obias,
            "rmask": rowmask[sl],
        })
    return in_maps


def round_f32r(x):
    """Round fp32 array to the fp32r grid (RNE to 11 mantissa bits)."""
    u = np.ascontiguousarray(x, np.float32).view(np.uint32)
    u = (u + 0x7FF + ((u >> 12) & 1)) & 0xFFFFF000
    return u.astype(np.uint32).view(np.float32)


_NC_CACHE = {}


def get_nc(sc=SC):
    if sc not in _NC_CACHE:
        _NC_CACHE[sc] = build(sc)
    return _NC_CACHE[sc]


def run(in_maps, trace=False):
    nc = get_nc(SC)
    return run_bass_kernel_spmd(nc, in_maps, list(range(NCORES)), trace=trace)


def kernel(**inputs):
    in_maps = host_prep(**inputs)
    res = run(in_maps).results
    out = np.concatenate([res[c]["out"] for c in range(NCORES)], axis=0)
    return np.ascontiguousarray(out.reshape(B, S, D))



# revision 7
# speedup vs baseline: 2.7577x; 2.7577x over previous
"""AttentionRPE kernel for 8 Trainium2 NeuronCores — bf16 single-pass design.

Math (per (b,s) row, T=128 targets, D=256, H=8 heads, DH=32, DR=32):
  q   = src @ Wsrc.T + bsrc                       [D]
  K'  = tgt @ Wk.T + rpe @ Rwk.T                  [T, D]
  att = softmax_h(q_h . K'_h / sqrt(DH))          [H, T]   (masked)
  out = (att @ V')_heads @ Wout.T + bout          [D]

Device formulation:
  * The whole q-path is linear in src, so qw = (q/sqrt(DH)) @ Wkx is folded
    ON HOST into per-row stationary vectors qk[(s,h), f] (f = 288 tgt|rpe
    features).  logits[(s,h), t] = sum_f qk[f,(s,h)] * tgtxT[f, t].
  * Host ships tgtx in BOTH layouts as bf16: transposed (txt, for the
    logits moving operand) and natural (txn, stationary for the G path).
    No on-chip transposes of the big tensor; all matmuls are single-pass
    bf16 (FWL fast-weight-load kicks in on 128-col stationaries).
  * Padding mask + off-window garbage masking folded into the 3rd logits
    matmul as 4 extra stationary rows (per-group one-hot selector).
  * G[f, j, h] = sum_t tgtx[t,f] * att[h,t] via natural-tgtx stationary +
    transposed-attention moving (8 cols/row).  Final: out = sum_k
    gall[:,k,:].T @ wfx[k] with host-folded wfx = (Wout_h @ Wvx_h).T.
  * All biases either cancel in softmax (K-side) or fold into the output
    bias (V-side).

Sharding: 1024 (b,s) rows split contiguously over 8 cores (128 each).
"""

import numpy as np
import ml_dtypes

import concourse.bass as bass
import concourse.bacc as bacc
import concourse.mybir as mybir
from concourse.tile import TileContext
from concourse.masks import make_identity
from concourse.bass_utils import run_bass_kernel_spmd

B, S, T, D = 2, 512, 128, 256
H, DH, DR = 8, 32, 32
DX = D + DR          # 288 = tgt|rpe feature dim
DOUT = D
NCORES = 8
BS = B * S           # 1024 total rows
SC = BS // NCORES    # 128 rows per core
NBLK = SC // 16      # 8 blocks of 16 rows
NGRP = SC // 4       # 32 groups of 4 rows

F32 = mybir.dt.float32
BF16 = mybir.dt.bfloat16
NPBF16 = np.dtype(ml_dtypes.bfloat16)

AX = mybir.AxisListType
ALU = mybir.AluOpType
ACTF = mybir.ActivationFunctionType

# f-chunk ranges for the G path / final projection.  c2 overlaps c1 so all
# three stationaries are full 128-col (FWL); wfx rows for the overlap are
# zeroed on host.
CH = [(0, 128), (128, 256), (160, 288)]


def build(sc=SC):
    assert sc % 16 == 0
    nblk = sc // 16
    nc = bacc.Bacc()

    txt_d = nc.dram_tensor("txt", [nblk * 4, 292, 512], BF16, kind="ExternalInput")
    txn_d = nc.dram_tensor("txn", [nblk, T, 16, DX], BF16, kind="ExternalInput")
    qk_d = nc.dram_tensor("qk", [292, sc, H], BF16, kind="ExternalInput")
    wfx_d = nc.dram_tensor("wfx", [24, 128, DOUT], BF16, kind="ExternalInput")
    obias_d = nc.dram_tensor("obias", [sc, DOUT], F32, kind="ExternalInput")
    rmask_d = nc.dram_tensor("rmask", [sc, 1], F32, kind="ExternalInput")
    out_d = nc.dram_tensor("out", [sc, DOUT], F32, kind="ExternalOutput")

    with TileContext(nc) as tc:
        with (
            tc.tile_pool(name="const", bufs=1) as cp,
            tc.tile_pool(name="txtp", bufs=6) as txtp,
            tc.tile_pool(name="txnp", bufs=3) as txnp,
            tc.tile_pool(name="attnp", bufs=2) as attnp,
            tc.tile_pool(name="smallp", bufs=2) as smallp,
            tc.tile_pool(name="ps_l", bufs=2, space="PSUM") as ps_l,
            tc.tile_pool(name="ps_at", bufs=2, space="PSUM") as ps_at,
            tc.tile_pool(name="ps_g", bufs=2, space="PSUM") as ps_g,
            tc.tile_pool(name="ps_o", bufs=1, space="PSUM") as ps_o,
        ):
            # ---------------- constants ----------------
            qk0 = cp.tile([128, sc, H], BF16, name="qk0")
            nc.sync.dma_start(out=qk0, in_=qk_d[0:128, :, :])
            qk1 = cp.tile([128, sc, H], BF16, name="qk1")
            nc.sync.dma_start(out=qk1, in_=qk_d[128:256, :, :])
            qk2 = cp.tile([36, sc, H], BF16, name="qk2")
            nc.sync.dma_start(out=qk2, in_=qk_d[256:292, :, :])
            qk0f = qk0.rearrange("p s h -> p (s h)")
            qk1f = qk1.rearrange("p s h -> p (s h)")
            qk2f = qk2.rearrange("p s h -> p (s h)")
            wfx = []
            for k in range(24):
                wt = cp.tile([128, DOUT], BF16, name=f"wfx{k}")
                nc.sync.dma_start(out=wt, in_=wfx_d[k, :, :])
                wfx.append(wt)
            eye = cp.tile([128, 128], F32, name="eye")
            make_identity(nc, eye)
            obias = cp.tile([sc, DOUT], F32, name="obias")
            nc.sync.dma_start(out=obias, in_=obias_d[:, :])
            rmask = cp.tile([sc, 1], F32, name="rmask")
            nc.sync.dma_start(out=rmask, in_=rmask_d[:, :])
            gall = cp.tile([128, 24, sc], BF16, name="gall")

            # ---------------- main loop ----------------
            for blk in range(nblk):
                txn = txnp.tile([T, 16, DX], BF16, tag="txn", name="txn")
                nc.sync.dma_start(out=txn, in_=txn_d[blk, :, :, :])

                l_ps = ps_l.tile([128, 512], F32, name="l_ps")
                for g4 in range(4):
                    g = blk * 4 + g4
                    t0 = txtp.tile([128, 512], BF16, tag="t0", name="t0")
                    nc.sync.dma_start(out=t0, in_=txt_d[g, 0:128, :])
                    t1 = txtp.tile([128, 512], BF16, tag="t1", name="t1")
                    nc.sync.dma_start(out=t1, in_=txt_d[g, 128:256, :])
                    t2 = txtp.tile([36, 512], BF16, tag="t2", name="t2")
                    nc.sync.dma_start(out=t2, in_=txt_d[g, 256:292, :])
                    osl = slice(g4 * 32, (g4 + 1) * 32)
                    csl = slice(g * 4 * H, (g + 1) * 4 * H)
                    nc.tensor.matmul(
                        l_ps[osl, :], qk0f[:, csl], t0,
                        start=True, stop=False, tile_position=(0, g4 * 32))
                    nc.tensor.matmul(
                        l_ps[osl, :], qk1f[:, csl], t1,
                        start=False, stop=False, tile_position=(0, g4 * 32))
                    nc.tensor.matmul(
                        l_ps[osl, :], qk2f[:, csl], t2,
                        start=False, stop=True, tile_position=(0, g4 * 32))

                # softmax over the 512-wide rows (off-window slots at -1e30)
                nmx = smallp.tile([128, 1], F32, tag="nmx", name="nmx")
                nc.vector.tensor_reduce(nmx, l_ps, axis=AX.X, op=ALU.max,
                                        negate=True)
                den = smallp.tile([128, 1], F32, tag="den", name="den")
                attn_e = attnp.tile([128, 512], F32, tag="ae", name="attn_e")
                nc.scalar.activation(attn_e, l_ps, ACTF.Exp, bias=nmx,
                                     scale=1.0, accum_out=den)
                rden = smallp.tile([128, 1], F32, tag="rden", name="rden")
                nc.vector.reciprocal(rden, den)
                attn_n = attnp.tile([128, 512], F32, tag="an", name="attn_n")
                nc.vector.tensor_scalar_mul(attn_n, attn_e, rden)

                # attn^T [t, (j,h)]: sum of 4 window transposes is exact
                # because off-window attn entries are exactly 0.
                at_ps = ps_at.tile([128, 128], F32, name="at_ps")
                for w in range(4):
                    nc.tensor.matmul(
                        at_ps, attn_n[:, w * 128:(w + 1) * 128], eye,
                        start=(w == 0), stop=(w == 3), is_transpose=True)
                atT = attnp.tile([128, 128], BF16, tag="atT", name="atT")
                nc.vector.tensor_copy(atT, at_ps)

                # G path: natural tgtx stationary (128-col, FWL), attT moving
                gt_ps = ps_g.tile([128, 24, 16], F32, name="gt_ps")
                for j in range(16):
                    av = atT[:, j * 8:(j + 1) * 8]
                    for c, (f0, f1) in enumerate(CH):
                        nc.tensor.matmul(
                            gt_ps[:, c * 8:(c + 1) * 8, j],
                            txn[:, j, f0:f1], av,
                            start=True, stop=True)
                bsl = slice(blk * 16, (blk + 1) * 16)
                nc.vector.tensor_copy(gall[:, :, bsl], gt_ps)

            # ---------------- output projection ----------------
            out_ps = ps_o.tile([sc, DOUT], F32, name="out_ps")
            for k in range(24):
                nc.tensor.matmul(
                    out_ps, gall[:, k, :], wfx[k],
                    start=(k == 0), stop=(k == 23))
            out_sb = cp.tile([sc, DOUT], F32, name="out_sb")
            nc.vector.tensor_tensor(out_sb, out_ps, obias, op=ALU.add)
            out_sb2 = cp.tile([sc, DOUT], F32, name="out_sb2")
            nc.vector.tensor_scalar_mul(out_sb2, out_sb, rmask)
            nc.sync.dma_start(out=out_d[:, :], in_=out_sb2)

    nc.finalize()
    return nc


def host_prep(src, tgt, rpe, tgt_padding_mask, in_proj_weight, in_proj_bias,
              out_proj_weight, out_proj_bias, rpe_weight, rpe_bias):
    """Host-side folding + layout prep.  Returns per-core input maps."""
    f = np.float32
    scale = f(1.0 / np.sqrt(DH))

    src_f = np.asarray(src, f).reshape(BS, D)
    ipw = np.asarray(in_proj_weight, f)
    ipb = np.asarray(in_proj_bias, f)
    opw = np.asarray(out_proj_weight, f)
    opb = np.asarray(out_proj_bias, f)
    rw = np.asarray(rpe_weight, f)
    rb = np.asarray(rpe_bias, f)

    # ---- q-path fold (host): qk[(f|rpe|sel), s, h] ----
    q_s = (src_f @ ipw[:D].T + ipb[:D]) * scale          # [BS, D]
    wk = ipw[D:2 * D]                                    # [e, d]
    rwk = rw[:D]                                         # [e, r]
    qh = q_s.reshape(BS, H, DH)
    qw = np.einsum('shk,hkf->shf', qh, wk.reshape(H, DH, D))     # [BS,H,D]
    qrw = np.einsum('shk,hkf->shf', qh, rwk.reshape(H, DH, DR))  # [BS,H,DR]
    sel = (np.arange(4)[:, None] == (np.arange(SC) % 4)[None, :])  # [4, SC]
    qk = np.empty((NCORES, 292, SC, H), NPBF16)
    qwT = qw.transpose(2, 0, 1)    # [D, BS, H]
    qrwT = qrw.transpose(2, 0, 1)  # [DR, BS, H]
    for c in range(NCORES):
        sl = slice(c * SC, (c + 1) * SC)
        qk[c, 0:D] = qwT[:, sl].astype(NPBF16)
        qk[c, D:DX] = qrwT[:, sl].astype(NPBF16)
        qk[c, DX:292] = np.broadcast_to(
            sel.astype(f)[:, :, None], (4, SC, H)).astype(NPBF16)

    # ---- tgtx in both layouts (bf16) ----
    tgtx = np.concatenate(
        [np.asarray(tgt, f).reshape(BS, T, D),
         np.asarray(rpe, f).reshape(BS, T, DR)], axis=-1)   # [BS, T, DX]
    tgtx16 = tgtx.astype(NPBF16)
    # natural per block: [core, blk, t, j, f]
    txn = np.ascontiguousarray(
        tgtx16.reshape(NCORES, NBLK, 16, T, DX).transpose(0, 1, 3, 2, 4))
    # transposed per group: [core, grp, f, j, t] (+4 mask rows)
    txtT = tgtx16.reshape(NCORES, NGRP, 4, T, DX).transpose(0, 1, 4, 2, 3)
    txt = np.empty((NCORES, NGRP, 292, 4, T), NPBF16)
    txt[:, :, 0:DX] = txtT

    # ---- mask rows: M[grp][m, j, t] = maskadd if j==m else -1e30 ----
    mask = np.asarray(tgt_padding_mask, bool).reshape(BS, T)
    no_valid = mask.all(-1)
    maskadd = np.where(mask & ~no_valid[:, None], f(-1e30), f(0.0))
    Mfull = np.full((BS, 4, T), -1e30, f).reshape(NCORES, NGRP, 4, 4, T)
    ma_g = maskadd.reshape(NCORES, NGRP, 4, T)
    for m in range(4):
        Mfull[:, :, m, m, :] = ma_g[:, :, m, :]
    txt[:, :, DX:292] = Mfull.astype(NPBF16)
    txt = txt.reshape(NCORES, NGRP, 292, 512)

    # ---- output-side folds ----
    wvx = np.concatenate([ipw[2 * D:3 * D], rw[D:2 * D]], axis=1)  # [e, DX]
    wfxh = np.empty((H, DX, DOUT), f)
    for h in range(H):
        hs = slice(h * DH, (h + 1) * DH)
        wfxh[h] = (opw[:, hs] @ wvx[hs, :]).T
    wfxk = np.zeros((24, 128, DOUT), f)
    for h in range(H):
        wfxk[h] = wfxh[h, CH[0][0]:CH[0][1]]
        wfxk[8 + h] = wfxh[h, CH[1][0]:CH[1][1]]
        wfxk[16 + h, 96:128] = wfxh[h, D:DX]     # rows 160:256 stay zero
    wfxk16 = wfxk.astype(NPBF16)

    obias = (opb + opw @ (ipb[2 * D:3 * D] + rb[D:2 * D]))[None, :]
    obias = np.ascontiguousarray(np.repeat(obias.astype(f), SC, axis=0))
    rowmask = np.ascontiguousarray((~no_valid).astype(f)[:, None])

    in_maps = []
    for c in range(NCORES):
        sl = slice(c * SC, (c + 1) * SC)
        in_maps.append({
            "txt": np.ascontiguousarray(txt[c]),
            "txn": np.ascontiguousarray(txn[c]),
            "qk": np.ascontiguousarray(qk[c]),
            "wfx": wfxk16,
            "obias": obias,
            "rmask": rowmask[sl],
        })
    return in_maps


_NC_CACHE = {}


def get_nc(sc=SC):
    if sc not in _NC_CACHE:
        _NC_CACHE[sc] = build(sc)
    return _NC_CACHE[sc]


def run(in_maps, trace=False):
    nc = get_nc(SC)
    return run_bass_kernel_spmd(nc, in_maps, list(range(NCORES)), trace=trace)


def kernel(**inputs):
    in_maps = host_prep(**inputs)
    res = run(in_maps).results
    out = np.concatenate([res[c]["out"] for c in range(NCORES)], axis=0)
    return np.ascontiguousarray(out.reshape(B, S, D))


# revision 8
# speedup vs baseline: 2.8884x; 1.0474x over previous
"""AttentionRPE kernel for 8 Trainium2 NeuronCores — bf16 single-pass design.

Math (per (b,s) row, T=128 targets, D=256, H=8 heads, DH=32, DR=32):
  q   = src @ Wsrc.T + bsrc                       [D]
  K'  = tgt @ Wk.T + rpe @ Rwk.T                  [T, D]
  att = softmax_h(q_h . K'_h / sqrt(DH))          [H, T]   (masked)
  out = (att @ V')_heads @ Wout.T + bout          [D]

Device formulation:
  * The whole q-path is linear in src, so qw = (q/sqrt(DH)) @ Wkx is folded
    ON HOST into per-row stationary vectors qk[(s,h), f] (f = 288 tgt|rpe
    features).  logits[(s,h), t] = sum_f qk[f,(s,h)] * tgtxT[f, t].
  * Host ships tgtx in BOTH layouts as bf16: transposed (for the logits
    moving operand) and natural (stationary for the G path).  No on-chip
    transposes of the big tensor; all matmuls are single-pass bf16 (FWL
    fast-weight-load kicks in on the 128-col stationaries).
  * Padding mask + off-window garbage masking folded into the 3rd logits
    matmul as 4 extra stationary rows (per-group one-hot selector).
  * G[f, j, h] = sum_t tgtx[t,f] * att[h,t] via natural-tgtx stationary +
    transposed-attention moving (8 cols/row).  Final: out = sum_k
    gall[:,k,:].T @ wfx[k] with host-folded wfx = (Wout_h @ Wvx_h).T.
  * All per-block data rides in 2 large DMAs (one per HWDGE engine) —
    HWDGE dispatch is ~0.7us/call on the issuing engine queue, so DMA
    count is minimized.

Sharding: 1024 (b,s) rows split contiguously over 8 cores (128 each).
"""

import numpy as np
import ml_dtypes

import concourse.bass as bass
import concourse.bacc as bacc
import concourse.mybir as mybir
from concourse.tile import TileContext
from concourse.masks import make_identity
from concourse.bass_utils import run_bass_kernel_spmd

B, S, T, D = 2, 512, 128, 256
H, DH, DR = 8, 32, 32
DX = D + DR          # 288 = tgt|rpe feature dim
DOUT = D
NCORES = 8
BS = B * S           # 1024 total rows
SC = BS // NCORES    # 128 rows per core
NBLK = SC // 16      # 8 blocks of 16 rows
NGRP = SC // 4       # 32 groups of 4 rows

F32 = mybir.dt.float32
BF16 = mybir.dt.bfloat16
NPBF16 = np.dtype(ml_dtypes.bfloat16)

AX = mybir.AxisListType
ALU = mybir.AluOpType
ACTF = mybir.ActivationFunctionType

# f-chunk ranges for the G path / final projection.  c2 overlaps c1 so all
# three stationaries are full 128-col (FWL); wfx rows for the overlap are
# zeroed on host.
CH = [(0, 128), (128, 256), (160, 288)]

# txb free-dim layout: [0:4608) natural (16j x 288f), [4608:6656) t0
# (4g x 512jt), [6656:8704) t1.
OFF_T0 = 16 * DX     # 4608
OFF_T1 = OFF_T0 + 2048


def build(sc=SC):
    assert sc % 16 == 0
    nblk = sc // 16
    nc = bacc.Bacc()

    txb_d = nc.dram_tensor("txb", [nblk, 128, 8704], BF16, kind="ExternalInput")
    tx2_d = nc.dram_tensor("tx2", [nblk, 36, 2048], BF16, kind="ExternalInput")
    qk01_d = nc.dram_tensor("qk01", [128, 2 * sc * H], BF16, kind="ExternalInput")
    qk2_d = nc.dram_tensor("qk2", [36, sc * H], BF16, kind="ExternalInput")
    wfx_d = nc.dram_tensor("wfx", [128, 24, DOUT], BF16, kind="ExternalInput")
    obias_d = nc.dram_tensor("obias", [sc, DOUT], F32, kind="ExternalInput")
    rmask_d = nc.dram_tensor("rmask", [sc, 1], F32, kind="ExternalInput")
    out_d = nc.dram_tensor("out", [sc, DOUT], F32, kind="ExternalOutput")

    with TileContext(nc) as tc:
        with (
            tc.tile_pool(name="const", bufs=1) as cp,
            tc.tile_pool(name="txbp", bufs=3) as txbp,
            tc.tile_pool(name="tx2p", bufs=3) as tx2p,
            tc.tile_pool(name="attnp", bufs=2) as attnp,
            tc.tile_pool(name="smallp", bufs=2) as smallp,
            tc.tile_pool(name="ps_l", bufs=2, space="PSUM") as ps_l,
            tc.tile_pool(name="ps_at", bufs=2, space="PSUM") as ps_at,
            tc.tile_pool(name="ps_g", bufs=2, space="PSUM") as ps_g,
            tc.tile_pool(name="ps_o", bufs=1, space="PSUM") as ps_o,
        ):
            # ---------------- constants ----------------
            qk01 = cp.tile([128, 2 * sc * H], BF16, name="qk01")
            nc.sync.dma_start(out=qk01, in_=qk01_d[:, :])
            qk2 = cp.tile([36, sc * H], BF16, name="qk2")
            nc.scalar.dma_start(out=qk2, in_=qk2_d[:, :])
            wfxt = cp.tile([128, 24, DOUT], BF16, name="wfxt")
            nc.scalar.dma_start(out=wfxt, in_=wfx_d[:, :, :])
            eye = cp.tile([128, 128], F32, name="eye")
            make_identity(nc, eye)
            obias = cp.tile([sc, DOUT], F32, name="obias")
            nc.scalar.dma_start(out=obias, in_=obias_d[:, :])
            rmask = cp.tile([sc, 1], F32, name="rmask")
            nc.scalar.dma_start(out=rmask, in_=rmask_d[:, :])
            gall = cp.tile([128, 24, sc], BF16, name="gall")

            # ---------------- main loop ----------------
            for blk in range(nblk):
                txb = txbp.tile([128, 8704], BF16, tag="txb", name="txb")
                nc.sync.dma_start(out=txb, in_=txb_d[blk, :, :])
                t2b = tx2p.tile([36, 2048], BF16, tag="t2b", name="t2b")
                nc.scalar.dma_start(out=t2b, in_=tx2_d[blk, :, :])

                l_ps = ps_l.tile([128, 512], F32, name="l_ps")
                for g4 in range(4):
                    g = blk * 4 + g4
                    csl = slice(g * 4 * H, (g + 1) * 4 * H)
                    c1sl = slice(sc * H + g * 4 * H, sc * H + (g + 1) * 4 * H)
                    osl = slice(g4 * 32, (g4 + 1) * 32)
                    tsl = slice(OFF_T0 + g4 * 512, OFF_T0 + (g4 + 1) * 512)
                    t1sl = slice(OFF_T1 + g4 * 512, OFF_T1 + (g4 + 1) * 512)
                    nc.tensor.matmul(
                        l_ps[osl, :], qk01[:, csl], txb[:, tsl],
                        start=True, stop=False, tile_position=(0, g4 * 32))
                    nc.tensor.matmul(
                        l_ps[osl, :], qk01[:, c1sl], txb[:, t1sl],
                        start=False, stop=False, tile_position=(0, g4 * 32))
                    nc.tensor.matmul(
                        l_ps[osl, :], qk2[:, csl],
                        t2b[:, g4 * 512:(g4 + 1) * 512],
                        start=False, stop=True, tile_position=(0, g4 * 32))

                # softmax over the 512-wide rows (off-window slots at -1e30)
                nmx = smallp.tile([128, 1], F32, tag="nmx", name="nmx")
                nc.vector.tensor_reduce(nmx, l_ps, axis=AX.X, op=ALU.max,
                                        negate=True)
                den = smallp.tile([128, 1], F32, tag="den", name="den")
                attn_e = attnp.tile([128, 512], F32, tag="ae", name="attn_e")
                nc.scalar.activation(attn_e, l_ps, ACTF.Exp, bias=nmx,
                                     scale=1.0, accum_out=den)
                rden = smallp.tile([128, 1], F32, tag="rden", name="rden")
                nc.vector.reciprocal(rden, den)
                attn_n = attnp.tile([128, 512], F32, tag="an", name="attn_n")
                nc.vector.tensor_scalar_mul(attn_n, attn_e, rden)

                # attn^T [t, (j,h)]: sum of 4 window transposes is exact
                # because off-window attn entries are exactly 0.
                at_ps = ps_at.tile([128, 128], F32, name="at_ps")
                for w in range(4):
                    nc.tensor.matmul(
                        at_ps, attn_n[:, w * 128:(w + 1) * 128], eye,
                        start=(w == 0), stop=(w == 3), is_transpose=True)
                atT = attnp.tile([128, 128], BF16, tag="atT", name="atT")
                nc.vector.tensor_copy(atT, at_ps)

                # G path: natural tgtx stationary (128-col, FWL), attT moving
                gt_ps = ps_g.tile([128, 24, 16], F32, name="gt_ps")
                for j in range(16):
                    av = atT[:, j * 8:(j + 1) * 8]
                    for c, (f0, f1) in enumerate(CH):
                        nc.tensor.matmul(
                            gt_ps[:, c * 8:(c + 1) * 8, j],
                            txb[:, j * DX + f0:j * DX + f1], av,
                            start=True, stop=True)
                bsl = slice(blk * 16, (blk + 1) * 16)
                nc.vector.tensor_copy(gall[:, :, bsl], gt_ps)

            # ---------------- output projection ----------------
            out_ps = ps_o.tile([sc, DOUT], F32, name="out_ps")
            for k in range(24):
                nc.tensor.matmul(
                    out_ps, gall[:, k, :], wfxt[:, k, :],
                    start=(k == 0), stop=(k == 23))
            out_sb = cp.tile([sc, DOUT], F32, name="out_sb")
            nc.vector.tensor_tensor(out_sb, out_ps, obias, op=ALU.add)
            out_sb2 = cp.tile([sc, DOUT], F32, name="out_sb2")
            nc.vector.tensor_scalar_mul(out_sb2, out_sb, rmask)
            nc.sync.dma_start(out=out_d[:, :], in_=out_sb2)

    nc.finalize()
    return nc


def host_prep(src, tgt, rpe, tgt_padding_mask, in_proj_weight, in_proj_bias,
              out_proj_weight, out_proj_bias, rpe_weight, rpe_bias):
    """Host-side folding + layout prep.  Returns per-core input maps."""
    f = np.float32
    scale = f(1.0 / np.sqrt(DH))

    src_f = np.asarray(src, f).reshape(BS, D)
    ipw = np.asarray(in_proj_weight, f)
    ipb = np.asarray(in_proj_bias, f)
    opw = np.asarray(out_proj_weight, f)
    opb = np.asarray(out_proj_bias, f)
    rw = np.asarray(rpe_weight, f)
    rb = np.asarray(rpe_bias, f)

    # ---- q-path fold (host): qk[(f|rpe|sel), s, h] ----
    q_s = (src_f @ ipw[:D].T + ipb[:D]) * scale          # [BS, D]
    wk = ipw[D:2 * D]                                    # [e, d]
    rwk = rw[:D]                                         # [e, r]
    qh = q_s.reshape(BS, H, DH)
    qw = np.einsum('shk,hkf->shf', qh, wk.reshape(H, DH, D))     # [BS,H,D]
    qrw = np.einsum('shk,hkf->shf', qh, rwk.reshape(H, DH, DR))  # [BS,H,DR]
    sel = (np.arange(4)[:, None] == (np.arange(SC) % 4)[None, :]).astype(f)
    qwT = qw.transpose(2, 0, 1).reshape(D, NCORES, SC * H)    # [D, c, s*h]
    qrwT = qrw.transpose(2, 0, 1).reshape(DR, NCORES, SC * H)
    qk01 = np.empty((NCORES, 128, 2 * SC * H), NPBF16)
    qk01[:, :, 0:SC * H] = qwT[0:128].transpose(1, 0, 2).astype(NPBF16)
    qk01[:, :, SC * H:] = qwT[128:256].transpose(1, 0, 2).astype(NPBF16)
    qk2 = np.empty((NCORES, 36, SC * H), NPBF16)
    qk2[:, 0:32] = qrwT.transpose(1, 0, 2).astype(NPBF16)
    selh = np.broadcast_to(sel[:, :, None], (4, SC, H)).reshape(4, SC * H)
    qk2[:, 32:36] = selh.astype(NPBF16)[None]

    # ---- tgtx in both layouts (bf16) ----
    tgtx = np.concatenate(
        [np.asarray(tgt, f).reshape(BS, T, D),
         np.asarray(rpe, f).reshape(BS, T, DR)], axis=-1)   # [BS, T, DX]
    tgtx16 = tgtx.astype(NPBF16)
    txb = np.empty((NCORES, NBLK, 128, 8704), NPBF16)
    # natural: txb[.., t, j*288+f]
    txb[:, :, :, 0:OFF_T0] = tgtx16.reshape(
        NCORES, NBLK, 16, T, DX).transpose(0, 1, 3, 2, 4).reshape(
        NCORES, NBLK, T, 16 * DX)
    # transposed: [c, blk, f, (g4, j, t)]
    txtT = tgtx16.reshape(NCORES, NBLK, 4, 4, T, DX).transpose(
        0, 1, 5, 2, 3, 4).reshape(NCORES, NBLK, DX, 2048)
    txb[:, :, :, OFF_T0:OFF_T1] = txtT[:, :, 0:128]
    txb[:, :, :, OFF_T1:8704] = txtT[:, :, 128:256]
    tx2 = np.empty((NCORES, NBLK, 36, 2048), NPBF16)
    tx2[:, :, 0:32] = txtT[:, :, 256:288]

    # ---- mask rows: M[m, (g4, j, t)] = maskadd if j==m else -1e30 ----
    mask = np.asarray(tgt_padding_mask, bool).reshape(BS, T)
    no_valid = mask.all(-1)
    maskadd = np.where(mask & ~no_valid[:, None], f(-1e30), f(0.0))
    Mfull = np.full((BS, 4, T), -1e30, f).reshape(NCORES, NBLK, 4, 4, 4, T)
    ma_g = maskadd.reshape(NCORES, NBLK, 4, 4, T)
    for m in range(4):
        Mfull[:, :, :, m, m, :] = ma_g[:, :, :, m, :]
    # Mfull dims: [c, blk, g4, m, j, t] -> [c, blk, m, (g4, j, t)]
    tx2[:, :, 32:36] = Mfull.transpose(0, 1, 3, 2, 4, 5).reshape(
        NCORES, NBLK, 4, 2048).astype(NPBF16)

    # ---- output-side folds ----
    wvx = np.concatenate([ipw[2 * D:3 * D], rw[D:2 * D]], axis=1)  # [e, DX]
    wfxh = np.empty((H, DX, DOUT), f)
    for h in range(H):
        hs = slice(h * DH, (h + 1) * DH)
        wfxh[h] = (opw[:, hs] @ wvx[hs, :]).T
    wfxk = np.zeros((24, 128, DOUT), f)
    for h in range(H):
        wfxk[h] = wfxh[h, CH[0][0]:CH[0][1]]
        wfxk[8 + h] = wfxh[h, CH[1][0]:CH[1][1]]
        wfxk[16 + h, 96:128] = wfxh[h, D:DX]     # rows 160:256 stay zero
    wfxk16 = np.ascontiguousarray(
        wfxk.transpose(1, 0, 2).astype(NPBF16))  # [128, 24, 256]

    obias = (opb + opw @ (ipb[2 * D:3 * D] + rb[D:2 * D]))[None, :]
    obias = np.ascontiguousarray(np.repeat(obias.astype(f), SC, axis=0))
    rowmask = np.ascontiguousarray((~no_valid).astype(f)[:, None])

    in_maps = []
    for c in range(NCORES):
        sl = slice(c * SC, (c + 1) * SC)
        in_maps.append({
            "txb": np.ascontiguousarray(txb[c]),
            "tx2": np.ascontiguousarray(tx2[c]),
            "qk01": np.ascontiguousarray(qk01[c]),
            "qk2": np.ascontiguousarray(qk2[c]),
            "wfx": wfxk16,
            "obias": obias,
            "rmask": rowmask[sl],
        })
    return in_maps


_NC_CACHE = {}


def get_nc(sc=SC):
    if sc not in _NC_CACHE:
        _NC_CACHE[sc] = build(sc)
    return _NC_CACHE[sc]


def run(in_maps, trace=False):
    nc = get_nc(SC)
    return run_bass_kernel_spmd(nc, in_maps, list(range(NCORES)), trace=trace)


def kernel(**inputs):
    in_maps = host_prep(**inputs)
    res = run(in_maps).results
    out = np.concatenate([res[c]["out"] for c in range(NCORES)], axis=0)
    return np.ascontiguousarray(out.reshape(B, S, D))


# revision 15
# speedup vs baseline: 3.2907x; 1.1393x over previous
"""AttentionRPE kernel for 8 Trainium2 NeuronCores — bf16 single-pass design.

Math (per (b,s) row, T=128 targets, D=256, H=8 heads, DH=32, DR=32):
  q   = src @ Wsrc.T + bsrc                       [D]
  K'  = tgt @ Wk.T + rpe @ Rwk.T                  [T, D]
  att = softmax_h(q_h . K'_h / sqrt(DH))          [H, T]   (masked)
  out = (att @ V')_heads @ Wout.T + bout          [D]

Device formulation:
  * The whole q-path is linear in src, so qw = (q/sqrt(DH)) @ Wkx is folded
    ON HOST into per-row stationary vectors qk[(s,h), f] (f = 288 tgt|rpe
    features).  logits[(s,h), t] = sum_f qk[f,(s,h)] * tgtxT[f, t].
  * Host ships tgtx in BOTH layouts as bf16: transposed (for the logits
    moving operand) and natural (stationary for the G path).  No on-chip
    transposes of the big tensor; all matmuls are single-pass bf16 (FWL
    fast-weight-load kicks in on the 128-col stationaries).
  * Padding mask + off-window garbage masking folded into the 3rd logits
    matmul as 4 extra stationary rows (per-group one-hot selector).
  * G[f, j, h] = sum_t tgtx[t,f] * att[h,t] via natural-tgtx stationary +
    transposed-attention moving (8 cols/row).  Final: out = sum_k
    gall[:,k,:].T @ wfx[k] with host-folded wfx = (Wout_h @ Wvx_h).T.
  * All per-block data rides in 2 large DMAs (one per HWDGE engine) —
    HWDGE dispatch is ~0.7us/call on the issuing engine queue, so DMA
    count is minimized.

Sharding: 1024 (b,s) rows split contiguously over 8 cores (128 each).
"""

import numpy as np
import ml_dtypes

import concourse.bass as bass
import concourse.bacc as bacc
import concourse.mybir as mybir
from concourse.tile import TileContext
from concourse.masks import make_identity
from concourse.bass_utils import run_bass_kernel_spmd

B, S, T, D = 2, 512, 128, 256
H, DH, DR = 8, 32, 32
DX = D + DR          # 288 = tgt|rpe feature dim
DOUT = D
NCORES = 8
BS = B * S           # 1024 total rows
SC = BS // NCORES    # 128 rows per core
NBLK = SC // 16      # 8 blocks of 16 rows
NGRP = SC // 4       # 32 groups of 4 rows

F32 = mybir.dt.float32
BF16 = mybir.dt.bfloat16
NPBF16 = np.dtype(ml_dtypes.bfloat16)

AX = mybir.AxisListType
ALU = mybir.AluOpType
ACTF = mybir.ActivationFunctionType

# f-chunk ranges for the G path / final projection.  c2 overlaps c1 so all
# three stationaries are full 128-col (FWL); wfx rows for the overlap are
# zeroed on host.
CH = [(0, 128), (128, 256), (160, 288)]

# txb free-dim layout: [0:4608) natural (16j x 288f), [4608:6656) t0
# (4g x 512jt), [6656:8704) t1.
OFF_T0 = 16 * DX     # 4608
OFF_T1 = OFF_T0 + 2048


def build(sc=SC):
    assert sc % 16 == 0
    nblk = sc // 16
    nc = bacc.Bacc()

    txt_d = nc.dram_tensor("txt", [nblk, 128, 4096], BF16, kind="ExternalInput")
    txn_d = nc.dram_tensor("txn", [nblk, 128, 4608], BF16, kind="ExternalInput")
    tx2_d = nc.dram_tensor("tx2", [nblk, 36, 2048], BF16, kind="ExternalInput")
    qk01_d = nc.dram_tensor("qk01", [128, 2 * sc * H], BF16, kind="ExternalInput")
    qk2_d = nc.dram_tensor("qk2", [36, sc * H], BF16, kind="ExternalInput")
    wfx_d = nc.dram_tensor("wfx", [128, 24, DOUT], BF16, kind="ExternalInput")
    obias_d = nc.dram_tensor("obias", [sc, DOUT], F32, kind="ExternalInput")
    rmask_d = nc.dram_tensor("rmask", [sc, 1], F32, kind="ExternalInput")
    out_d = nc.dram_tensor("out", [sc, DOUT], F32, kind="ExternalOutput")

    with TileContext(nc) as tc:
        with (
            tc.tile_pool(name="const", bufs=1) as cp,
            tc.tile_pool(name="txtp", bufs=4) as txtp,
            tc.tile_pool(name="txnp", bufs=4) as txnp,
            tc.tile_pool(name="tx2p", bufs=4) as tx2p,
            tc.tile_pool(name="attnp", bufs=2) as attnp,
            tc.tile_pool(name="smallp", bufs=2) as smallp,
            tc.tile_pool(name="ps_l", bufs=2, space="PSUM") as ps_l,
            tc.tile_pool(name="ps_at", bufs=2, space="PSUM") as ps_at,
            tc.tile_pool(name="ps_g", bufs=2, space="PSUM") as ps_g,
            tc.tile_pool(name="ps_o", bufs=1, space="PSUM") as ps_o,
        ):
            # ---------------- constants ----------------
            qk01 = cp.tile([128, 2 * sc * H], BF16, name="qk01")
            nc.sync.dma_start(out=qk01, in_=qk01_d[:, :])
            qk2 = cp.tile([36, sc * H], BF16, name="qk2")
            nc.gpsimd.dma_start(out=qk2, in_=qk2_d[:, :])
            wfxt = cp.tile([128, 24, DOUT], BF16, name="wfxt")
            nc.gpsimd.dma_start(out=wfxt, in_=wfx_d[:, :, :])
            eye = cp.tile([128, 128], F32, name="eye")
            make_identity(nc, eye)
            obias = cp.tile([sc, DOUT], F32, name="obias")
            nc.gpsimd.dma_start(out=obias, in_=obias_d[:, :])
            rmask = cp.tile([sc, 1], F32, name="rmask")
            nc.gpsimd.dma_start(out=rmask, in_=rmask_d[:, :])
            gall = cp.tile([128, 24, sc], BF16, name="gall")

            # ---------------- main loop ----------------
            for blk in range(nblk):
                txt = txtp.tile([128, 4096], BF16, tag="txt", name="txt")
                nc.sync.dma_start(out=txt, in_=txt_d[blk, :, :])
                txn = txnp.tile([128, 4608], BF16, tag="txn", name="txn")
                nc.scalar.dma_start(out=txn, in_=txn_d[blk, :, :])
                t2b = tx2p.tile([36, 2048], BF16, tag="t2b", name="t2b")
                nc.gpsimd.dma_start(out=t2b, in_=tx2_d[blk, :, :])

                l_ps = ps_l.tile([128, 512], F32, name="l_ps")
                for g4 in range(4):
                    g = blk * 4 + g4
                    csl = slice(g * 4 * H, (g + 1) * 4 * H)
                    c1sl = slice(sc * H + g * 4 * H, sc * H + (g + 1) * 4 * H)
                    osl = slice(g4 * 32, (g4 + 1) * 32)
                    nc.tensor.matmul(
                        l_ps[osl, :], qk01[:, csl],
                        txt[:, g4 * 512:(g4 + 1) * 512],
                        start=True, stop=False, tile_position=(0, g4 * 32))
                    nc.tensor.matmul(
                        l_ps[osl, :], qk01[:, c1sl],
                        txt[:, 2048 + g4 * 512:2048 + (g4 + 1) * 512],
                        start=False, stop=False, tile_position=(0, g4 * 32))
                    nc.tensor.matmul(
                        l_ps[osl, :], qk2[:, csl],
                        t2b[:, g4 * 512:(g4 + 1) * 512],
                        start=False, stop=True, tile_position=(0, g4 * 32))

                # softmax over the 512-wide rows (off-window slots at -1e30)
                nmx = smallp.tile([128, 1], F32, tag="nmx", name="nmx")
                nc.vector.tensor_reduce(nmx, l_ps, axis=AX.X, op=ALU.max,
                                        negate=True)
                den = smallp.tile([128, 1], F32, tag="den", name="den")
                attn_e = attnp.tile([128, 512], F32, tag="ae", name="attn_e")
                nc.scalar.activation(attn_e, l_ps, ACTF.Exp, bias=nmx,
                                     scale=1.0, accum_out=den)
                rden = smallp.tile([128, 1], F32, tag="rden", name="rden")
                nc.vector.reciprocal(rden, den)
                attn_n = attnp.tile([128, 512], F32, tag="an", name="attn_n")
                nc.vector.tensor_scalar_mul(attn_n, attn_e, rden)

                # attn^T [t, (j,h)]: sum of 4 window transposes is exact
                # because off-window attn entries are exactly 0.
                at_ps = ps_at.tile([128, 128], F32, name="at_ps")
                for w in range(4):
                    nc.tensor.matmul(
                        at_ps, attn_n[:, w * 128:(w + 1) * 128], eye,
                        start=(w == 0), stop=(w == 3), is_transpose=True)
                atT = attnp.tile([128, 128], BF16, tag="atT", name="atT")
                nc.vector.tensor_copy(atT, at_ps)

                # G path: natural tgtx stationary (128-col, FWL), attT moving
                gt_ps = ps_g.tile([128, 24, 16], F32, name="gt_ps")
                for j in range(16):
                    av = atT[:, j * 8:(j + 1) * 8]
                    for c, (f0, f1) in enumerate(CH):
                        nc.tensor.matmul(
                            gt_ps[:, c * 8:(c + 1) * 8, j],
                            txn[:, j * DX + f0:j * DX + f1], av,
                            start=True, stop=True)
                bsl = slice(blk * 16, (blk + 1) * 16)
                nc.vector.tensor_copy(gall[:, :, bsl], gt_ps)

            # ---------------- output projection ----------------
            out_ps = ps_o.tile([sc, DOUT], F32, name="out_ps")
            for k in range(24):
                nc.tensor.matmul(
                    out_ps, gall[:, k, :], wfxt[:, k, :],
                    start=(k == 0), stop=(k == 23))
            out_sb = cp.tile([sc, DOUT], F32, name="out_sb")
            nc.vector.tensor_tensor(out_sb, out_ps, obias, op=ALU.add)
            out_sb2 = cp.tile([sc, DOUT], F32, name="out_sb2")
            nc.vector.tensor_scalar_mul(out_sb2, out_sb, rmask)
            nc.sync.dma_start(out=out_d[:, :], in_=out_sb2)

    nc.finalize()
    return nc


def host_prep(src, tgt, rpe, tgt_padding_mask, in_proj_weight, in_proj_bias,
              out_proj_weight, out_proj_bias, rpe_weight, rpe_bias):
    """Host-side folding + layout prep.  Returns per-core input maps."""
    f = np.float32
    scale = f(1.0 / np.sqrt(DH))

    src_f = np.asarray(src, f).reshape(BS, D)
    ipw = np.asarray(in_proj_weight, f)
    ipb = np.asarray(in_proj_bias, f)
    opw = np.asarray(out_proj_weight, f)
    opb = np.asarray(out_proj_bias, f)
    rw = np.asarray(rpe_weight, f)
    rb = np.asarray(rpe_bias, f)

    # ---- q-path fold (host): qk[(f|rpe|sel), s, h] ----
    q_s = (src_f @ ipw[:D].T + ipb[:D]) * scale          # [BS, D]
    wk = ipw[D:2 * D]                                    # [e, d]
    rwk = rw[:D]                                         # [e, r]
    qh = q_s.reshape(BS, H, DH)
    qw = np.einsum('shk,hkf->shf', qh, wk.reshape(H, DH, D))     # [BS,H,D]
    qrw = np.einsum('shk,hkf->shf', qh, rwk.reshape(H, DH, DR))  # [BS,H,DR]
    sel = (np.arange(4)[:, None] == (np.arange(SC) % 4)[None, :]).astype(f)
    qwT = qw.transpose(2, 0, 1).reshape(D, NCORES, SC * H)    # [D, c, s*h]
    qrwT = qrw.transpose(2, 0, 1).reshape(DR, NCORES, SC * H)
    qk01 = np.empty((NCORES, 128, 2 * SC * H), NPBF16)
    qk01[:, :, 0:SC * H] = qwT[0:128].transpose(1, 0, 2).astype(NPBF16)
    qk01[:, :, SC * H:] = qwT[128:256].transpose(1, 0, 2).astype(NPBF16)
    qk2 = np.empty((NCORES, 36, SC * H), NPBF16)
    qk2[:, 0:32] = qrwT.transpose(1, 0, 2).astype(NPBF16)
    selh = np.broadcast_to(sel[:, :, None], (4, SC, H)).reshape(4, SC * H)
    qk2[:, 32:36] = selh.astype(NPBF16)[None]

    # ---- tgtx in both layouts (bf16) ----
    tgtx = np.concatenate(
        [np.asarray(tgt, f).reshape(BS, T, D),
         np.asarray(rpe, f).reshape(BS, T, DR)], axis=-1)   # [BS, T, DX]
    tgtx16 = tgtx.astype(NPBF16)
    # natural: txn[.., t, j*288+f]
    txn = np.ascontiguousarray(tgtx16.reshape(
        NCORES, NBLK, 16, T, DX).transpose(0, 1, 3, 2, 4).reshape(
        NCORES, NBLK, T, 16 * DX))
    # transposed: [c, blk, f, (g4, j, t)]
    txtT = tgtx16.reshape(NCORES, NBLK, 4, 4, T, DX).transpose(
        0, 1, 5, 2, 3, 4).reshape(NCORES, NBLK, DX, 2048)
    txt = np.empty((NCORES, NBLK, 128, 4096), NPBF16)
    txt[:, :, :, 0:2048] = txtT[:, :, 0:128]
    txt[:, :, :, 2048:4096] = txtT[:, :, 128:256]
    tx2 = np.empty((NCORES, NBLK, 36, 2048), NPBF16)
    tx2[:, :, 0:32] = txtT[:, :, 256:288]

    # ---- mask rows: M[m, (g4, j, t)] = maskadd if j==m else -1e30 ----
    mask = np.asarray(tgt_padding_mask, bool).reshape(BS, T)
    no_valid = mask.all(-1)
    maskadd = np.where(mask & ~no_valid[:, None], f(-1e30), f(0.0))
    Mfull = np.full((BS, 4, T), -1e30, f).reshape(NCORES, NBLK, 4, 4, 4, T)
    ma_g = maskadd.reshape(NCORES, NBLK, 4, 4, T)
    for m in range(4):
        Mfull[:, :, :, m, m, :] = ma_g[:, :, :, m, :]
    # Mfull dims: [c, blk, g4, m, j, t] -> [c, blk, m, (g4, j, t)]
    tx2[:, :, 32:36] = Mfull.transpose(0, 1, 3, 2, 4, 5).reshape(
        NCORES, NBLK, 4, 2048).astype(NPBF16)

    # ---- output-side folds ----
    wvx = np.concatenate([ipw[2 * D:3 * D], rw[D:2 * D]], axis=1)  # [e, DX]
    wfxh = np.empty((H, DX, DOUT), f)
    for h in range(H):
        hs = slice(h * DH, (h + 1) * DH)
        wfxh[h] = (opw[:, hs] @ wvx[hs, :]).T
    wfxk = np.zeros((24, 128, DOUT), f)
    for h in range(H):
        wfxk[h] = wfxh[h, CH[0][0]:CH[0][1]]
        wfxk[8 + h] = wfxh[h, CH[1][0]:CH[1][1]]
        wfxk[16 + h, 96:128] = wfxh[h, D:DX]     # rows 160:256 stay zero
    wfxk16 = np.ascontiguousarray(
        wfxk.transpose(1, 0, 2).astype(NPBF16))  # [128, 24, 256]

    obias = (opb + opw @ (ipb[2 * D:3 * D] + rb[D:2 * D]))[None, :]
    obias = np.ascontiguousarray(np.repeat(obias.astype(f), SC, axis=0))
    rowmask = np.ascontiguousarray((~no_valid).astype(f)[:, None])

    in_maps = []
    for c in range(NCORES):
        sl = slice(c * SC, (c + 1) * SC)
        in_maps.append({
            "txt": np.ascontiguousarray(txt[c]),
            "txn": np.ascontiguousarray(txn[c]),
            "tx2": np.ascontiguousarray(tx2[c]),
            "qk01": np.ascontiguousarray(qk01[c]),
            "qk2": np.ascontiguousarray(qk2[c]),
            "wfx": wfxk16,
            "obias": obias,
            "rmask": rowmask[sl],
        })
    return in_maps


_NC_CACHE = {}


def get_nc(sc=SC):
    if sc not in _NC_CACHE:
        _NC_CACHE[sc] = build(sc)
    return _NC_CACHE[sc]


def run(in_maps, trace=False):
    nc = get_nc(SC)
    return run_bass_kernel_spmd(nc, in_maps, list(range(NCORES)), trace=trace)


def kernel(**inputs):
    in_maps = host_prep(**inputs)
    res = run(in_maps).results
    out = np.concatenate([res[c]["out"] for c in range(NCORES)], axis=0)
    return np.ascontiguousarray(out.reshape(B, S, D))


# revision 16
# speedup vs baseline: 4.2845x; 1.3020x over previous
"""AttentionRPE kernel for 8 Trainium2 NeuronCores — bf16 single-pass design.

Math (per (b,s) row, T=128 targets, D=256, H=8 heads, DH=32, DR=32):
  q   = src @ Wsrc.T + bsrc                       [D]
  K'  = tgt @ Wk.T + rpe @ Rwk.T                  [T, D]
  att = softmax_h(q_h . K'_h / sqrt(DH))          [H, T]   (masked)
  out = (att @ V')_heads @ Wout.T + bout          [D]

Device formulation:
  * The whole q-path is linear in src, so qw = (q/sqrt(DH)) @ Wkx is folded
    ON HOST into per-row stationary vectors qk[(s,h), f] (f = 288 tgt|rpe
    features).  logits[(s,h), t] = sum_f qk[f,(s,h)] * tgtxT[f, t].
  * Host ships tgtx in BOTH layouts as bf16: transposed (for the logits
    moving operand) and natural (stationary for the G path).  No on-chip
    transposes of the big tensor; all matmuls are single-pass bf16 (FWL
    fast-weight-load kicks in on the 128-col stationaries).
  * Padding mask + off-window garbage masking folded into the 3rd logits
    matmul as 4 extra stationary rows (per-group one-hot selector).
  * G[f, j, h] = sum_t tgtx[t,f] * att[h,t] via natural-tgtx stationary +
    transposed-attention moving (8 cols/row).  Final: out = sum_k
    gall[:,k,:].T @ wfx[k] with host-folded wfx = (Wout_h @ Wvx_h).T.
  * All per-block data rides in 2 large DMAs (one per HWDGE engine) —
    HWDGE dispatch is ~0.7us/call on the issuing engine queue, so DMA
    count is minimized.

Sharding: 1024 (b,s) rows split contiguously over 8 cores (128 each).
"""

import numpy as np
import ml_dtypes

import concourse.bass as bass
import concourse.bacc as bacc
import concourse.mybir as mybir
from concourse.tile import TileContext
from concourse.masks import make_identity
from concourse.bass_utils import run_bass_kernel_spmd

B, S, T, D = 2, 512, 128, 256
H, DH, DR = 8, 32, 32
DX = D + DR          # 288 = tgt|rpe feature dim
DOUT = D
NCORES = 8
BS = B * S           # 1024 total rows
SC = BS // NCORES    # 128 rows per core
NBLK = SC // 16      # 8 blocks of 16 rows
NGRP = SC // 4       # 32 groups of 4 rows

F32 = mybir.dt.float32
BF16 = mybir.dt.bfloat16
F8E3 = mybir.dt.float8e3
NPBF16 = np.dtype(ml_dtypes.bfloat16)
NPF8E3 = np.dtype(ml_dtypes.float8_e3m4)

AX = mybir.AxisListType
ALU = mybir.AluOpType
ACTF = mybir.ActivationFunctionType

# f-chunk ranges for the G path / final projection.  c2 overlaps c1 so all
# three stationaries are full 128-col (FWL); wfx rows for the overlap are
# zeroed on host.
CH = [(0, 128), (128, 256), (160, 288)]

# txb free-dim layout: [0:4608) natural (16j x 288f), [4608:6656) t0
# (4g x 512jt), [6656:8704) t1.
OFF_T0 = 16 * DX     # 4608
OFF_T1 = OFF_T0 + 2048


def build(sc=SC):
    assert sc % 16 == 0
    nblk = sc // 16
    nc = bacc.Bacc()

    txt_d = nc.dram_tensor("txt", [nblk, 128, 4096], BF16, kind="ExternalInput")
    txn_d = nc.dram_tensor("txn", [nblk, 128, 4608], F8E3, kind="ExternalInput")
    tx2_d = nc.dram_tensor("tx2", [nblk, 36, 2048], BF16, kind="ExternalInput")
    qk01_d = nc.dram_tensor("qk01", [128, 2 * sc * H], BF16, kind="ExternalInput")
    qk2_d = nc.dram_tensor("qk2", [36, sc * H], BF16, kind="ExternalInput")
    wfx_d = nc.dram_tensor("wfx", [128, 24, DOUT], BF16, kind="ExternalInput")
    obias_d = nc.dram_tensor("obias", [sc, DOUT], F32, kind="ExternalInput")
    rmask_d = nc.dram_tensor("rmask", [sc, 1], F32, kind="ExternalInput")
    out_d = nc.dram_tensor("out", [sc, DOUT], F32, kind="ExternalOutput")

    with TileContext(nc) as tc:
        with (
            tc.tile_pool(name="const", bufs=1) as cp,
            tc.tile_pool(name="txtp", bufs=4) as txtp,
            tc.tile_pool(name="txnp", bufs=4) as txnp,
            tc.tile_pool(name="tx2p", bufs=4) as tx2p,
            tc.tile_pool(name="attnp", bufs=2) as attnp,
            tc.tile_pool(name="smallp", bufs=2) as smallp,
            tc.tile_pool(name="ps_l", bufs=2, space="PSUM") as ps_l,
            tc.tile_pool(name="ps_at", bufs=2, space="PSUM") as ps_at,
            tc.tile_pool(name="ps_g", bufs=2, space="PSUM") as ps_g,
            tc.tile_pool(name="ps_o", bufs=1, space="PSUM") as ps_o,
        ):
            # ---------------- constants ----------------
            qk01 = cp.tile([128, 2 * sc * H], BF16, name="qk01")
            nc.gpsimd.dma_start(out=qk01, in_=qk01_d[:, :])
            qk2 = cp.tile([36, sc * H], BF16, name="qk2")
            nc.gpsimd.dma_start(out=qk2, in_=qk2_d[:, :])
            eye = cp.tile([128, 128], F32, name="eye")
            make_identity(nc, eye)
            obias = cp.tile([sc, DOUT], F32, name="obias")
            nc.gpsimd.dma_start(out=obias, in_=obias_d[:, :])
            rmask = cp.tile([sc, 1], F32, name="rmask")
            nc.gpsimd.dma_start(out=rmask, in_=rmask_d[:, :])
            gall = cp.tile([128, 24, sc], BF16, name="gall")

            # ---------------- main loop ----------------
            for blk in range(nblk):
                txt = txtp.tile([128, 4096], BF16, tag="txt", name="txt")
                nc.sync.dma_start(out=txt[:, 0:2048], in_=txt_d[blk, :, 0:2048])
                nc.sync.dma_start(out=txt[:, 2048:4096], in_=txt_d[blk, :, 2048:4096])
                txn = txnp.tile([128, 4608], F8E3, tag="txn", name="txn")
                nc.scalar.dma_start(out=txn, in_=txn_d[blk, :, :])
                t2b = tx2p.tile([36, 2048], BF16, tag="t2b", name="t2b")
                nc.gpsimd.dma_start(out=t2b, in_=tx2_d[blk, :, :])

                l_ps = ps_l.tile([128, 512], F32, name="l_ps")
                for g4 in range(4):
                    g = blk * 4 + g4
                    csl = slice(g * 4 * H, (g + 1) * 4 * H)
                    c1sl = slice(sc * H + g * 4 * H, sc * H + (g + 1) * 4 * H)
                    osl = slice(g4 * 32, (g4 + 1) * 32)
                    nc.tensor.matmul(
                        l_ps[osl, :], qk01[:, csl],
                        txt[:, g4 * 512:(g4 + 1) * 512],
                        start=True, stop=False, tile_position=(0, g4 * 32))
                    nc.tensor.matmul(
                        l_ps[osl, :], qk01[:, c1sl],
                        txt[:, 2048 + g4 * 512:2048 + (g4 + 1) * 512],
                        start=False, stop=False, tile_position=(0, g4 * 32))
                    nc.tensor.matmul(
                        l_ps[osl, :], qk2[:, csl],
                        t2b[:, g4 * 512:(g4 + 1) * 512],
                        start=False, stop=True, tile_position=(0, g4 * 32))

                # softmax over the 512-wide rows (off-window slots at -1e30)
                nmx = smallp.tile([128, 1], F32, tag="nmx", name="nmx")
                nc.vector.tensor_reduce(nmx, l_ps, axis=AX.X, op=ALU.max,
                                        negate=True)
                den = smallp.tile([128, 1], F32, tag="den", name="den")
                attn_e = attnp.tile([128, 512], F32, tag="ae", name="attn_e")
                nc.scalar.activation(attn_e, l_ps, ACTF.Exp, bias=nmx,
                                     scale=1.0, accum_out=den)
                rden = smallp.tile([128, 1], F32, tag="rden", name="rden")
                nc.vector.reciprocal(rden, den)
                attn_n = attnp.tile([128, 512], F32, tag="an", name="attn_n")
                nc.vector.tensor_scalar_mul(attn_n, attn_e, rden)

                # attn^T [t, (j,h)]: sum of 4 window transposes is exact
                # because off-window attn entries are exactly 0.
                at_ps = ps_at.tile([128, 128], F32, name="at_ps")
                for w in range(4):
                    nc.tensor.matmul(
                        at_ps, attn_n[:, w * 128:(w + 1) * 128], eye,
                        start=(w == 0), stop=(w == 3), is_transpose=True)
                atT = attnp.tile([128, 128], BF16, tag="atT", name="atT")
                nc.vector.tensor_copy(atT, at_ps)

                # G path: natural tgtx stationary (128-col, FWL), attT moving
                gt_ps = ps_g.tile([128, 24, 16], F32, name="gt_ps")
                for j in range(16):
                    av = atT[:, j * 8:(j + 1) * 8]
                    for c, (f0, f1) in enumerate(CH):
                        nc.tensor.matmul(
                            gt_ps[:, c * 8:(c + 1) * 8, j],
                            txn[:, j * DX + f0:j * DX + f1], av,
                            start=True, stop=True)
                bsl = slice(blk * 16, (blk + 1) * 16)
                nc.vector.tensor_copy(gall[:, :, bsl], gt_ps)

            # ---------------- output projection ----------------
            wfxt = cp.tile([128, 24, DOUT], BF16, name="wfxt")
            nc.gpsimd.dma_start(out=wfxt, in_=wfx_d[:, :, :])
            out_ps = ps_o.tile([sc, DOUT], F32, name="out_ps")
            for k in range(24):
                nc.tensor.matmul(
                    out_ps, gall[:, k, :], wfxt[:, k, :],
                    start=(k == 0), stop=(k == 23))
            out_sb = cp.tile([sc, DOUT], F32, name="out_sb")
            nc.vector.tensor_tensor(out_sb, out_ps, obias, op=ALU.add)
            out_sb2 = cp.tile([sc, DOUT], F32, name="out_sb2")
            nc.vector.tensor_scalar_mul(out_sb2, out_sb, rmask)
            nc.sync.dma_start(out=out_d[:, :], in_=out_sb2)

    nc.finalize()
    return nc


def host_prep(src, tgt, rpe, tgt_padding_mask, in_proj_weight, in_proj_bias,
              out_proj_weight, out_proj_bias, rpe_weight, rpe_bias):
    """Host-side folding + layout prep.  Returns per-core input maps."""
    f = np.float32
    scale = f(1.0 / np.sqrt(DH))

    src_f = np.asarray(src, f).reshape(BS, D)
    ipw = np.asarray(in_proj_weight, f)
    ipb = np.asarray(in_proj_bias, f)
    opw = np.asarray(out_proj_weight, f)
    opb = np.asarray(out_proj_bias, f)
    rw = np.asarray(rpe_weight, f)
    rb = np.asarray(rpe_bias, f)

    # ---- q-path fold (host): qk[(f|rpe|sel), s, h] ----
    q_s = (src_f @ ipw[:D].T + ipb[:D]) * scale          # [BS, D]
    wk = ipw[D:2 * D]                                    # [e, d]
    rwk = rw[:D]                                         # [e, r]
    qh = q_s.reshape(BS, H, DH)
    qw = np.einsum('shk,hkf->shf', qh, wk.reshape(H, DH, D))     # [BS,H,D]
    qrw = np.einsum('shk,hkf->shf', qh, rwk.reshape(H, DH, DR))  # [BS,H,DR]
    sel = (np.arange(4)[:, None] == (np.arange(SC) % 4)[None, :]).astype(f)
    qwT = qw.transpose(2, 0, 1).reshape(D, NCORES, SC * H)    # [D, c, s*h]
    qrwT = qrw.transpose(2, 0, 1).reshape(DR, NCORES, SC * H)
    qk01 = np.empty((NCORES, 128, 2 * SC * H), NPBF16)
    qk01[:, :, 0:SC * H] = qwT[0:128].transpose(1, 0, 2).astype(NPBF16)
    qk01[:, :, SC * H:] = qwT[128:256].transpose(1, 0, 2).astype(NPBF16)
    qk2 = np.empty((NCORES, 36, SC * H), NPBF16)
    qk2[:, 0:32] = qrwT.transpose(1, 0, 2).astype(NPBF16)
    selh = np.broadcast_to(sel[:, :, None], (4, SC, H)).reshape(4, SC * H)
    qk2[:, 32:36] = selh.astype(NPBF16)[None]

    # ---- tgtx in both layouts (bf16) ----
    tgtx = np.concatenate(
        [np.asarray(tgt, f).reshape(BS, T, D),
         np.asarray(rpe, f).reshape(BS, T, DR)], axis=-1)   # [BS, T, DX]
    tgtx16 = tgtx.astype(NPBF16)
    # natural: txn[.., t, j*288+f] (fp8 e3m4 for the G path)
    txn = np.ascontiguousarray(tgtx.reshape(
        NCORES, NBLK, 16, T, DX).transpose(0, 1, 3, 2, 4).reshape(
        NCORES, NBLK, T, 16 * DX).astype(NPF8E3))
    # transposed: [c, blk, f, (g4, j, t)]
    txtT = tgtx16.reshape(NCORES, NBLK, 4, 4, T, DX).transpose(
        0, 1, 5, 2, 3, 4).reshape(NCORES, NBLK, DX, 2048)
    txt = np.empty((NCORES, NBLK, 128, 4096), NPBF16)
    txt[:, :, :, 0:2048] = txtT[:, :, 0:128]
    txt[:, :, :, 2048:4096] = txtT[:, :, 128:256]
    tx2 = np.empty((NCORES, NBLK, 36, 2048), NPBF16)
    tx2[:, :, 0:32] = txtT[:, :, 256:288]

    # ---- mask rows: M[m, (g4, j, t)] = maskadd if j==m else -1e30 ----
    mask = np.asarray(tgt_padding_mask, bool).reshape(BS, T)
    no_valid = mask.all(-1)
    maskadd = np.where(mask & ~no_valid[:, None], f(-1e30), f(0.0))
    Mfull = np.full((BS, 4, T), -1e30, f).reshape(NCORES, NBLK, 4, 4, 4, T)
    ma_g = maskadd.reshape(NCORES, NBLK, 4, 4, T)
    for m in range(4):
        Mfull[:, :, :, m, m, :] = ma_g[:, :, :, m, :]
    # Mfull dims: [c, blk, g4, m, j, t] -> [c, blk, m, (g4, j, t)]
    tx2[:, :, 32:36] = Mfull.transpose(0, 1, 3, 2, 4, 5).reshape(
        NCORES, NBLK, 4, 2048).astype(NPBF16)

    # ---- output-side folds ----
    wvx = np.concatenate([ipw[2 * D:3 * D], rw[D:2 * D]], axis=1)  # [e, DX]
    wfxh = np.empty((H, DX, DOUT), f)
    for h in range(H):
        hs = slice(h * DH, (h + 1) * DH)
        wfxh[h] = (opw[:, hs] @ wvx[hs, :]).T
    wfxk = np.zeros((24, 128, DOUT), f)
    for h in range(H):
        wfxk[h] = wfxh[h, CH[0][0]:CH[0][1]]
        wfxk[8 + h] = wfxh[h, CH[1][0]:CH[1][1]]
        wfxk[16 + h, 96:128] = wfxh[h, D:DX]     # rows 160:256 stay zero
    wfxk16 = np.ascontiguousarray(
        wfxk.transpose(1, 0, 2).astype(NPBF16))  # [128, 24, 256]

    obias = (opb + opw @ (ipb[2 * D:3 * D] + rb[D:2 * D]))[None, :]
    obias = np.ascontiguousarray(np.repeat(obias.astype(f), SC, axis=0))
    rowmask = np.ascontiguousarray((~no_valid).astype(f)[:, None])

    in_maps = []
    for c in range(NCORES):
        sl = slice(c * SC, (c + 1) * SC)
        in_maps.append({
            "txt": np.ascontiguousarray(txt[c]),
            "txn": np.ascontiguousarray(txn[c]),
            "tx2": np.ascontiguousarray(tx2[c]),
            "qk01": np.ascontiguousarray(qk01[c]),
            "qk2": np.ascontiguousarray(qk2[c]),
            "wfx": wfxk16,
            "obias": obias,
            "rmask": rowmask[sl],
        })
    return in_maps


_NC_CACHE = {}


def get_nc(sc=SC):
    if sc not in _NC_CACHE:
        _NC_CACHE[sc] = build(sc)
    return _NC_CACHE[sc]


def run(in_maps, trace=False):
    nc = get_nc(SC)
    return run_bass_kernel_spmd(nc, in_maps, list(range(NCORES)), trace=trace)


def kernel(**inputs):
    in_maps = host_prep(**inputs)
    res = run(in_maps).results
    out = np.concatenate([res[c]["out"] for c in range(NCORES)], axis=0)
    return np.ascontiguousarray(out.reshape(B, S, D))
